# revision 26
# baseline (speedup 1.0000x reference)
"""Trainium2 Bass kernel: pilot-based channel estimator (LS + linear blend).

Problem structure (hardcoded from the reference):
  Nfft = 4194304 subcarriers, pilots every SPACING=16 -> P = 262144 pilots at
  positions 16*k.  Per-pilot LS estimate H[k] = weights[k] * (Y[16k] / Xp[k])
  (complex).  One extrapolated pilot H[P] is appended at position Nfft-1.
  Every output sample idx = 16*k + j blends:
      out_r = alpha*Hr[k+1] + beta*Hr[k] + gamma*(j/16)
      out_i = alpha*Hi[k+1] + beta*Hi[k]
  except the final group (k = P-1) which uses ramp j/15 and the extrapolated
  right pilot.  Output is [Nfft, 2] fp32 (real, imag interleaved).

Sharding: the output axis is split across 8 cores (sequence parallel); the
small pilot arrays (Xp, weights, and the pilot samples of Y -- a host-side
strided slice, per the replicate-small-pilot-arrays sharding hint) are packed
into one per-core input.  No cross-core communication.

Schedule (v8, ~14.3us/core vs the 27.75us dense baseline):
  * Input is packed host-side into 4 per-chunk blocks: block k holds columns
    [64k, 64k+64] of [xr|xi|yr|yi], contiguous per partition (1040B), so each
    64-group chunk loads in its own 0.37us DMA above the 512B element
    threshold (input DMA ~1.5us total instead of 11.7us for dense Y).
  * The device writes the output in bf16 (rounding ~2^-9 relative, far
    inside the 2e-2 accuracy budget), halving the store traffic that
    dominates the DMA stream (5.83us of ~7.3us priced DMA).
  * Per-chunk chain: q/sq + den + numerators (DVE/ACT/Pool per SCHED), DVE
    reciprocal, fused scalar_tensor_tensor Hp = beta*w*num/den written as
    interleaved bf16, then the blend AI = Hp[g] + (alpha/beta)*Hp[g+1] as a
    flat packed-bf16 tensor_tensor ADD (DVE 2x mode).
  * The 16x expansion is one broadcast tensor_copy per output chunk:
    all-bf16 packed operands hit the DVE 4x performance mode (0.26ns/elem);
    two mid-stream chunks run on the otherwise idle ACT engine.
  * tile_wait_until gates (GATES) keep later chunks' DVE ops out of the
    greedy list scheduler's view of an earlier chunk's critical path, so the
    store stream anchors at ~6.6us and runs gap-free.
  * The end-of-spectrum extrapolated pilot affects only the final 16 output
    samples of the whole problem; those are patched on host.
"""

import numpy as np

import concourse.bass as bass
import concourse.bacc as bacc
import concourse.mybir as mybir
from concourse import tile
from concourse.bass_utils import run_bass_kernel_spmd

FP32 = mybir.dt.float32
BF16 = mybir.dt.bfloat16
NPART = 128
SPACING = 16
NCORES = 8
GPP = 256                    # pilot groups per partition
NP = GPP + 1                 # pilot columns per partition (incl. right bound)
YCOLS = GPP * SPACING        # 4096 Y samples per partition
X_LEN = NPART * GPP + 1           # per-segment input length (32769)
OUT_LEN = NPART * GPP * 32        # output elements per core (1048576)

# The per-core input is packed host-side into per-chunk blocks: block k holds
# columns [64k, 64k+64] (65 cols, one overlap col) of all four segments
# [xr | xi | yr | yi], contiguous per partition (260 floats = 1040B, so the
# chunk loads stay above the 512B DMA element threshold).  Chunks are fully
# independent: each computes its own 65 Hp columns and 64 AI groups.
# (first group, ngroups) per chunk; block k packs cols [g0, g0+ng] (ng+1)
CHUNKS = [(0, 64), (64, 64), (128, 64), (192, 64)]
# gate_ms: scheduler eligibility for each chunk's DVE ops (see tile_wait_until)
GATES = [0.0, 0.0046, 0.0058, 0.0070]
# expansion chunks: (first group, ngroups, engine); nest within chunks.
# engine "f" = fused TT-ADD straight from Hp (skips the AI dependency; only
# valid when ratio == 1, falls back to "v" otherwise)
EXP_CHUNKS = [(0, 24, "v"), (24, 40, "v"), (64, 40, "v"), (104, 24, "a"),
              (128, 40, "v"), (168, 24, "a"), (192, 40, "v"), (232, 24, "v")]
# per-chunk engine map: sq=x^2 path ("a" splits Square to ACT), den/numr/t2/numi
SCHED = [dict(sq="a", den="v", numr="v", t2="p", numi="p")] + [
    dict(sq="a", den="v", numr="p", t2="p", numi="p") for _ in range(3)]

ADD = mybir.AluOpType.add
SUB = mybir.AluOpType.subtract
MUL = mybir.AluOpType.mult


def _ap(view, dims):
    """Replace the free dims of a [p, 1]-column AP view with custom dims."""
    return bass.AP(tensor=view.tensor, offset=view.offset,
                   ap=[list(view.ap[0])] + [list(d) for d in dims])


def _dview(handle, offset, dims):
    a = handle.ap()
    return bass.AP(tensor=a.tensor, offset=offset, ap=[list(d) for d in dims])


def build_nc(ratio, beta, use_w=False):
    """Single-core Bass program (same NEFF runs on all cores).

    ratio = alpha/beta, beta: blend immediates (Hp = beta*w*num/den).
    use_w: multiply pilot estimates by the weights vector (fifth segment).
    """
    nseg = 5 if use_w else 4
    ratio = float(ratio)
    beta = float(beta)
    nc = bacc.Bacc(trn_type="TRN2", debug=False)
    total = sum(ng + 1 for _, ng in CHUNKS) * NPART * nseg
    xw_in = nc.dram_tensor("xw", [total], FP32, kind="ExternalInput")
    out_d = nc.dram_tensor("out", [OUT_LEN], BF16, kind="ExternalOutput")

    with tile.TileContext(nc) as tc:
        with (
            tc.tile_pool(name="persist", bufs=1) as pp,
            tc.tile_pool(name="chunk", bufs=2) as cp,
            tc.tile_pool(name="opool", bufs=4) as op_,
        ):
            nsegb = 5 if use_w else 4
            loads = []
            off = 0
            for k, (g0, ng) in enumerate(CHUNKS):
                blk = nsegb * (ng + 1)
                t = pp.tile([NPART, blk], FP32, name=f"xw{k}", tag=f"xw{k}")
                nc.sync.dma_start(
                    out=t, in_=_dview(xw_in, off, [[blk, NPART], [1, blk]]))
                loads.append(t)
                off += NPART * blk

            Hp = pp.tile([NPART, 2 * NP], BF16)   # interleaved beta*w*H
            AI = pp.tile([NPART, 2 * GPP], BF16)  # interleaved (re, im) blend

            exp_done = 0
            for k, (c0, ng) in enumerate(CHUNKS):
                xs = loads[k]
                C = ng + 1
                L = C
                x2 = _ap(xs[:, 0:1], [[L, 2], [1, C]])
                y2 = _ap(xs[:, 2 * L:2 * L + 1], [[L, 2], [1, C]])
                wv = xs[:, 4 * L:4 * L + C] if use_w else None

                import contextlib

                def gate(gate_ms=GATES[k]):
                    return (contextlib.nullcontext() if gate_ms == 0
                            else tc.tile_wait_until(gate_ms))

                sch = SCHED[k]
                eng = {"v": nc.vector, "p": nc.gpsimd, "a": nc.scalar}
                den = cp.tile([NPART, C], FP32, tag="den", name=f"den{k}")
                num = cp.tile([NPART, 2 * C], FP32, tag="num", name=f"num{k}")
                t2a = cp.tile([NPART, 2 * C], FP32, tag="t2a", name=f"t2a{k}")
                rec = cp.tile([NPART, C], FP32, tag="rec", name=f"rec{k}")
                if sch["sq"] == "a":
                    # ACT squares x; DVE does only t1 = y*x
                    sq = cp.tile([NPART, 2 * C], FP32, tag="sq",
                                 name=f"sq{k}")
                    nc.scalar.activation(
                        out=sq, in_=x2,
                        func=mybir.ActivationFunctionType.Square)
                    t1 = cp.tile([NPART, 2 * C], FP32, tag="t1",
                                 name=f"t1{k}")
                    with gate():
                        eng[sch.get("t1", "v")].tensor_tensor(
                            out=t1, in0=y2, in1=x2, op=MUL)
                    qd, qn = sq, t1
                    qoff = 0
                else:
                    # q = [xr|xi|yr|yi] * [xr|xi|xr|xi] in one op
                    q = cp.tile([NPART, 4 * C], FP32, tag="q", name=f"q{k}")
                    with gate():
                        eng[sch.get("t1", "v")].tensor_tensor(
                            out=q,
                            in0=_ap(xs[:, 0:1],
                                    [[2 * L, 2], [L, 2], [1, C]]),
                            in1=_ap(xs[:, 0:1], [[0, 2], [L, 2], [1, C]]),
                            op=MUL)
                    qd, qn = q, q
                    qoff = 2 * C
                with gate():
                    eng[sch["den"]].tensor_tensor(
                        out=den, in0=qd[:, 0:C], in1=qd[:, C:2 * C], op=ADD)
                    eng[sch["numr"]].tensor_tensor(
                        out=num[:, 0:C], in0=qn[:, qoff:qoff + C],
                        in1=qn[:, qoff + C:qoff + 2 * C], op=ADD)
                    nc.vector.reciprocal(out=rec, in_=den)
                # t2 = [yr|yi] * [xi|xr] in one op (negative seg stride)
                eng[sch["t2"]].tensor_tensor(
                    out=_ap(t2a[:, 0:1], [[C, 2], [1, C]]),
                    in0=y2,
                    in1=_ap(xs[:, L:L + 1], [[-L, 2], [1, C]]),
                    op=MUL)
                eng[sch["numi"]].tensor_tensor(out=num[:, C:2 * C],
                                               in0=t2a[:, C:2 * C],
                                               in1=t2a[:, 0:C], op=SUB)
                with gate():
                    if use_w:
                        wrec = cp.tile([NPART, C], FP32, tag="wrec",
                                       name=f"wrec{k}")
                        nc.vector.tensor_tensor(out=wrec, in0=rec, in1=wv,
                                                op=MUL)
                        rec = wrec
                    # Hp[2*(c0+t)+s] = beta * num * rec  (interleaved bf16)
                    nc.vector.scalar_tensor_tensor(
                        out=_ap(Hp[:, 2 * c0:2 * c0 + 1], [[2, C], [1, 2]]),
                        in0=_ap(num[:, 0:1], [[1, C], [C, 2]]),
                        scalar=beta,
                        in1=_ap(rec[:, 0:1], [[1, C], [0, 2]]),
                        op0=MUL, op1=MUL)

                    # blend AI[2g+s] = Hp[2g+s] + ratio*Hp[2g+2+s]: flat
                    # packed bf16 (tensor_tensor ADD hits the DVE 2x mode)
                    g0 = c0
                    G = ng
                    if ratio == 1.0:
                        nc.vector.tensor_tensor(
                            out=_ap(AI[:, 2 * g0:2 * g0 + 1], [[1, 2 * G]]),
                            in0=_ap(Hp[:, 2 * g0:2 * g0 + 1], [[1, 2 * G]]),
                            in1=_ap(Hp[:, 2 * g0 + 2:2 * g0 + 3],
                                    [[1, 2 * G]]),
                            op=ADD)
                    else:
                        nc.vector.scalar_tensor_tensor(
                            out=_ap(AI[:, 2 * g0:2 * g0 + 1], [[1, 2 * G]]),
                            in0=_ap(Hp[:, 2 * g0 + 2:2 * g0 + 3],
                                    [[1, 2 * G]]),
                            scalar=ratio,
                            in1=_ap(Hp[:, 2 * g0:2 * g0 + 1], [[1, 2 * G]]),
                            op0=MUL, op1=ADD)

                    # expansion (4x bf16 broadcast copy) + stores
                    while exp_done < len(EXP_CHUNKS):
                        e0, G_, ee = EXP_CHUNKS[exp_done]
                        if e0 + G_ > g0 + G:
                            break
                        out_t = op_.tile([NPART, 32 * G_], BF16, tag=f"o{G_}",
                                         name=f"o{e0}")
                        eo = _ap(out_t[:, 0:1], [[32, G_], [2, 16], [1, 2]])
                        ei = _ap(AI[:, 2 * e0:2 * e0 + 1],
                                 [[2, G_], [0, 16], [1, 2]])
                        if ee == "f" and ratio == 1.0:
                            nc.vector.tensor_tensor(
                                out=eo,
                                in0=_ap(Hp[:, 2 * e0:2 * e0 + 1],
                                        [[2, G_], [0, 16], [1, 2]]),
                                in1=_ap(Hp[:, 2 * e0 + 2:2 * e0 + 3],
                                        [[2, G_], [0, 16], [1, 2]]),
                                op=ADD)
                        elif ee == "a":
                            nc.scalar.copy(out=eo, in_=ei)
                        else:
                            eng[ee if ee != "f" else "v"].tensor_copy(
                                out=eo, in_=ei)
                        nc.sync.dma_start(
                            out=_dview(out_d, e0 * 32,
                                       [[32 * GPP, NPART], [1, 32 * G_]]),
                            in_=out_t)
                        exp_done += 1
    nc.compile()
    return nc


# ---------------------------------------------------------------- host side --

def make_core_inputs(c, Y_real, Y_imag, Xp_real, Xp_imag, weights, use_w):
    f32 = np.float32
    ypc = NPART * YCOLS            # 524288 Y samples per core per component
    gpc = NPART * GPP              # 32768 pilots per core
    y0 = c * ypc
    k0 = c * gpc
    if c == NCORES - 1:
        yr = np.concatenate([Y_real[y0::SPACING], np.zeros(1, f32)])
        yi = np.concatenate([Y_imag[y0::SPACING], np.zeros(1, f32)])
        xr = np.concatenate([Xp_real[k0:k0 + gpc], np.ones(1, f32)])
        xi = np.concatenate([Xp_imag[k0:k0 + gpc], np.zeros(1, f32)])
        ww = np.concatenate([weights[k0:k0 + gpc], np.ones(1, f32)])
    else:
        yr = Y_real[y0:y0 + ypc + 1:SPACING]
        yi = Y_imag[y0:y0 + ypc + 1:SPACING]
        xr = Xp_real[k0:k0 + X_LEN]
        xi = Xp_imag[k0:k0 + X_LEN]
        ww = weights[k0:k0 + X_LEN]
    segs = [np.ascontiguousarray(xr, f32), np.ascontiguousarray(xi, f32),
            np.ascontiguousarray(yr, f32), np.ascontiguousarray(yi, f32)]
    if use_w:
        segs.append(np.ascontiguousarray(ww, f32))
    # pack into per-chunk blocks: block k, partition p holds columns
    # [g0, g0+ng] of every segment, contiguous (see kernel layout notes)
    nsegb = len(segs)
    parts = []
    for (g0, ng) in CHUNKS:
        blk = np.empty((NPART, nsegb, ng + 1), f32)
        idx = (np.arange(NPART)[:, None] * GPP
               + g0 + np.arange(ng + 1)[None, :])
        for s, seg in enumerate(segs):
            blk[:, s, :] = seg[idx]
        parts.append(blk.reshape(-1))
    return {"xw": np.concatenate(parts)}


def _numpy_fallback(Y_real, Y_imag, Xp_real, Xp_imag, weights, alpha, beta,
                    gamma, pilot_pos, Nfft):
    """Exact port of the reference for unexpected input structure."""
    Yr = Y_real[pilot_pos]
    Yi = Y_imag[pilot_pos]
    den = Xp_real * Xp_real + Xp_imag * Xp_imag
    LSr = (Yr * Xp_real + Yi * Xp_imag) / den
    LSi = (Yi * Xp_real - Yr * Xp_imag) / den
    Hr = LSr * weights
    Hi = LSi * weights
    loc = pilot_pos.astype(np.float32)
    dx = loc[-1] - loc[-2]
    slope_r = (Hr[-1] - Hr[-2]) / dx
    slope_i = (Hi[-1] - Hi[-2]) / dx
    d_end = np.float32(Nfft - 1) - loc[-1]
    Hr = np.concatenate([Hr, Hr[-1:] + slope_r * d_end])
    Hi = np.concatenate([Hi, Hi[-1:] + slope_i * d_end])
    loc = np.concatenate([loc, np.array([Nfft - 1], np.float32)])
    idx = np.arange(Nfft, dtype=np.float32)
    left = np.clip(np.searchsorted(loc, idx, side="right") - 1, 0,
                   loc.shape[0] - 2)
    right = left + 1
    X0 = loc[left]
    X1 = loc[right]
    df = np.where(X1 - X0 > 0, (idx - X0) / (X1 - X0), np.float32(0.0))
    out_r = alpha * Hr[right] + beta * Hr[left] + gamma * df
    out_i = alpha * Hi[right] + beta * Hi[left]
    return np.stack([out_r, out_i], axis=-1).astype(np.float32)


_NC_CACHE = {}


def _get_nc(ratio, beta, use_w):
    key = (float(ratio), float(beta), use_w)
    if key not in _NC_CACHE:
        _NC_CACHE[key] = build_nc(ratio, beta, use_w)
    return _NC_CACHE[key]


def run_sharded(Y_real, Y_imag, Xp_real, Xp_imag, weights, alpha, beta,
                use_w, trace=False):
    ratio = float(alpha) / float(beta)
    nc = _get_nc(ratio, float(beta), use_w)
    in_maps = [
        make_core_inputs(c, Y_real, Y_imag, Xp_real, Xp_imag, weights, use_w)
        for c in range(NCORES)
    ]
    res = run_bass_kernel_spmd(nc, in_maps, core_ids=list(range(NCORES)),
                               trace=trace)
    out = np.concatenate([np.asarray(r["out"]) for r in res.results])
    return out.astype(np.float32).reshape(-1, 2), res


def kernel(**inputs):
    f32 = np.float32
    Y_real = np.asarray(inputs["Y_real"], f32)
    Y_imag = np.asarray(inputs["Y_imag"], f32)
    Xp_real = np.asarray(inputs["Xp_real"], f32)
    Xp_imag = np.asarray(inputs["Xp_imag"], f32)
    weights = np.asarray(inputs["weights"], f32)
    alpha = f32(np.asarray(inputs["alpha"]))
    beta = f32(np.asarray(inputs["beta"]))
    gamma = f32(np.asarray(inputs["gamma"]))
    pilot_pos = np.asarray(inputs["pilot_pos"])
    Nfft = int(np.asarray(inputs["Nfft"]))

    P = NCORES * NPART * GPP
    ok = (Nfft == NCORES * NPART * GPP * SPACING
          and Y_real.shape == (Nfft,) and Y_imag.shape == (Nfft,)
          and Xp_real.shape == (P,) and Xp_imag.shape == (P,)
          and weights.shape == (P,) and pilot_pos.shape == (P,)
          and gamma == f32(0.0) and beta != f32(0.0)
          and np.array_equal(pilot_pos,
                             np.arange(P, dtype=np.int64) * SPACING))
    if not ok:
        # unexpected structure -> bit-exact host fallback
        return _numpy_fallback(Y_real, Y_imag, Xp_real, Xp_imag, weights,
                               alpha, beta, gamma, pilot_pos, Nfft)

    use_w = not bool(np.all(weights == f32(1.0)))
    out, _ = run_sharded(Y_real, Y_imag, Xp_real, Xp_imag, weights, alpha,
                         beta, use_w=use_w)

    # ---- host boundary patch: the extrapolated end pilot only affects the
    # final 16 output samples of the whole spectrum ----
    den2 = Xp_real[-2:] ** 2 + Xp_imag[-2:] ** 2
    Yr2 = Y_real[pilot_pos[-2:]]
    Yi2 = Y_imag[pilot_pos[-2:]]
    Hr2 = (Yr2 * Xp_real[-2:] + Yi2 * Xp_imag[-2:]) / den2 * weights[-2:]
    Hi2 = (Yi2 * Xp_real[-2:] - Yr2 * Xp_imag[-2:]) / den2 * weights[-2:]
    hr_ext = Hr2[1] + (Hr2[1] - Hr2[0]) * f32(15.0 / 16.0)
    hi_ext = Hi2[1] + (Hi2[1] - Hi2[0]) * f32(15.0 / 16.0)
    out[Nfft - 16:, 0] = alpha * hr_ext + beta * Hr2[1]
    out[Nfft - 16:, 1] = alpha * hi_ext + beta * Hi2[1]
    return out
